# revision 41
# baseline (speedup 1.0000x reference)
"""Trainium2 Bass kernel for CompositionalPINN forward.

Reference semantics (B=262144, H=256, N_STEPS=8):
    state = state_dz[:, :4]; qop = state_dz[:, 4:5]; dz_sub = state_dz[:, 5:6]/8
    n_full = floor(z_frac*8); frac = z_frac*8 - n_full
    for step in range(8):
        state += (n_full > step) * MLP(state, qop, dz_sub)        # residual MLP
    state += (frac > 1e-6) * MLP(state, qop, frac*dz_sub)
    MLP(x) = silu(silu(silu(x@W1+b1)@W2+b2)@W3+b3)@W4+b4  (6->256->256->256->4)

Strategy: pure data parallel over 8 cores.  Host transposes inputs to a
feature-major layout and sorts samples by n_full (descending, dealt
round-robin across cores) so each NTILE-sample tile only runs
max(n_full)+1 MLP evals instead of 9.  Device program is layer-major
over GROUP interleaved tiles so the PE sees a dense back-to-back matmul
stream (keeps the HAM clock-gate warm) while ACT computes silu for the
previous layer.

Per eval (tile of NTILE=512 samples on one of GROUP=4 pipeline lanes,
each lane owning a 2-bank psum slot; lanes staggered by OFFSET stages):
  - L1: 2 matmuls  K=8 f32r (state,qop,dz[,dz_partial]) -> psum [128,1024]
  - silu on ACT in one [128,1024] instruction -> fp16 activations
    (same 10-bit mantissa as the f32r path, but fp16 streams the PE
    moving operand at full rate)
  - L2/L3: 4 fp16 matmuls each (K=128 x2 accumulate, M=128 x2)
  - L4: 2 fp16 matmuls -> delta psum [4,512] (in the lane's slot)
  - state += delta in-place in f32r (one DVE add; ~tf32 rounding per
    update costs ~4e-3 final scale-relative error vs the 2e-2 gate, and
    keeps the saturated Scalar engine free of mirror casts).
    Boundary-tile masked evals multiply the delta by host-precomputed
    4-row broadcast masks.

A custom-DVE 3-op exp-squaring silu path (SA env knob) exists to
offload part of each silu from ACT to the Vector engine; measured
slower in this pipeline (DVE FIFO head-of-line blocking), so disabled
by default.
"""

import numpy as np
from contextlib import ExitStack

import concourse.bass as bass
import concourse.tile as tile
from concourse import bacc, mybir
from concourse.bass_utils import run_bass_kernel_spmd

F32 = mybir.dt.float32
F32R = mybir.dt.float32r
F16 = mybir.dt.float16
Silu = mybir.ActivationFunctionType.Silu

NCORES = 8
NTILE = 512
GROUP = 4                           # pipeline lanes (psum: GROUP x 2-bank slots)
CHUNK_TILES = 8                     # tiles per DMA chunk
H = 256
NSTEPS = 8
MM = 512                            # matmul moving-dim chunk (fp32 max / psum bank)

# silu split: ACT handles columns [0:SA) of the [128, 2*NTILE] layer,
# custom-DVE ops handle [SA:2*NTILE).  SA = 2*NTILE disables the DVE path.
SA_DEFAULT = 896   # ACT/DVE balance point: DVE 3-op tail on 128 cols

# xm row layout (row 7 is a zero spare; its L1 weight rows are zero)
R_QOP = 4
R_DZSUB = 5
R_DZPART = 6

# ---- custom-DVE silu tail: silu(x) = relu(x) - |x|*sigma(-|x|) with
# sigma(-m) ~= E*p(E), E = q^32, q = 1 - m*(A0 - m*A1) (minimax-fitted
# e^{-m/32} seed), p(E) = P0 - E*(P1 - P2*E).  Max |silu error| 3.5e-4
# for |x| up to 40 (pre-activations here stay within [-9, 8]).
SILU_A0 = 0.029486568953419258
SILU_A1 = 0.00023859243894259991
SILU_P0 = 0.8829550403348707
SILU_P1 = 0.544980283163726
SILU_P2 = 0.1606029741431314

_DVE_OPS = {}


def _register_dve_ops():
    """Append the three silu-tail ops to concourse's custom-DVE registry.
    This is the documented extension point (dve_ops.OPS) applied at
    runtime; shas are computed from lower() output so they always match."""
    global _DVE_OPS
    if _DVE_OPS:
        return _DVE_OPS
    import concourse.dve_ops as dve_ops
    from concourse.dve_spec import Spec, lower, maxx, sq, Zero, One, Src0, Src1, C0, C1, C2
    from concourse.dve_spec import _has_src1
    from concourse.dve_uop import DveOpSpec

    def silu_q4_ref(in0, in1, c0, c1, c2):
        m = np.abs(in0.astype(np.float32))
        q = 1.0 - m * (c0 - m * c1)
        return (q * q * q * q).astype(np.float32)

    def silu_ep_ref(in0, in1, c0, c1, c2):
        e = in0.astype(np.float32) ** 8
        p = c0 - e * (c1 - c2 * e)
        return (e * p).astype(np.float32)

    def silu_fin_ref(in0, in1, c0, c1, c2):
        x = in1.astype(np.float32)
        return (np.maximum(x, 0.0) - np.abs(x) * in0.astype(np.float32)).astype(
            np.float32
        )

    m0 = maxx(Src0, Zero - Src0)
    q = One - m0 * (C0 - m0 * C1)
    spec_q4 = Spec(body=sq(sq(q)), reference=silu_q4_ref)

    e = sq(sq(sq(Src0)))
    spec_ep = Spec(body=e * (C0 - e * (C1 - C2 * e)), reference=silu_ep_ref)

    m1 = maxx(Src1, Zero - Src1)
    spec_fin = Spec(body=maxx(Src1, Zero) - m1 * Src0, reference=silu_fin_ref)

    ops = {}
    for name, spec in [
        ("SILU_Q4_ANT", spec_q4),
        ("SILU_EP_ANT", spec_ep),
        ("SILU_FIN_ANT", spec_fin),
    ]:
        existing = [op for op in dve_ops.OPS if op.name == name]
        if existing:
            ops[name] = existing[0]
            continue
        row = dve_ops._CUSTOM_DVE_ROW_BASE + len(dve_ops.OPS)
        assert row < 0x20
        dve_ops._SUB_OPCODE_FOR_NAME[name] = row
        shas = {}
        for ver in ("v3", "v4"):
            try:
                uops = lower(spec, ver=ver)
                shas[ver] = DveOpSpec(
                    name=name, opcode=row, uops=uops, rd1_en=_has_src1(spec)
                ).sha(ver)
            except Exception:
                pass
        op = dve_ops.DveOp(name, spec, subdim=False, uops_sha=shas)
        dve_ops.OPS.append(op)
        dve_ops.CUSTOM_DVE_SPECS[name] = spec
        ops[name] = op
    _DVE_OPS = ops
    return ops


OFFSET = 1          # slot offset between adjacent pipeline lanes


def _emit_slots(schedule, tiles):
    """Yield (tile, eval_idx, eval_desc, stage) in device-emission order.

    Tiles are dealt round-robin onto GROUP pipeline lanes; each eval is
    4 stages (L1+silu, L2+silu, L3+silu, L4+add).  Lane p lags lane p-1
    by OFFSET stages, so one tile's serial L4->add->cast tail always
    overlaps another tile's dense mid-layer matmuls and the PE stream
    never develops the ~1-2us/eval gap that re-throttles the HAM clock
    gate.  Used by both the program builder and the host mask packer --
    must stay identical."""
    lanes = [[] for _ in range(GROUP)]
    for t in range(tiles):
        lane = lanes[t % GROUP]
        for e, ev in enumerate(schedule[t]):
            for stage in range(4):
                lane.append((t, e, ev, stage))
    maxlen = max(len(l) for l in lanes) if lanes else 0
    for k in range(maxlen + OFFSET * (GROUP - 1)):
        for p in range(GROUP):
            i = k - OFFSET * p
            if 0 <= i < len(lanes[p]):
                yield lanes[p][i]


_BUILD_CACHE = {}

LAST_EXEC_NS = None  # set when BASSK_TRACE=1


def _install_ntff_hook():
    """The agent image lacks antenv.axon_hooks; synthesize it so
    run_bass_kernel_spmd(trace=True) can reach the NTFF profiler."""
    import sys
    import types
    if "antenv.axon_hooks" in sys.modules:
        return True
    try:
        import antenv
        from trn_agent_boot.trn_boot import _ntff_profile_via_ctypes
        hook = _ntff_profile_via_ctypes("/opt/axon/libaxon_pjrt.so")
        if hook is None:
            return False
        mod = types.ModuleType("antenv.axon_hooks")
        mod.get_axon_ntff_profile_hook = lambda: hook
        mod.set_axon_ntff_profile_hook = lambda h: None
        sys.modules["antenv.axon_hooks"] = mod
        antenv.axon_hooks = mod
        return True
    except Exception:
        return False


def _build(schedule, use_bias, n_core, sa):
    """schedule: tuple over tiles of tuples of (is_partial, use_mask)."""
    tiles = n_core // NTILE
    n_masked = sum(1 for tev in schedule for (_, m) in tev if m)
    W2N = 2 * NTILE
    if sa < W2N:
        ops = _register_dve_ops()
    nc = bacc.Bacc("TRN2", target_bir_lowering=False, debug=False,
                   num_devices=NCORES)

    xm_d = nc.declare_dram_parameter("xm", [8, n_core], F32R, isOutput=False)
    mk_d = nc.declare_dram_parameter("mask4", [4, max(1, n_masked) * NTILE],
                                     F32, isOutput=False)
    w1_d = nc.declare_dram_parameter("w1", [8, 512], F32R, isOutput=False)
    w2_d = nc.declare_dram_parameter("w2", [128, 512], F16, isOutput=False)
    w3_d = nc.declare_dram_parameter("w3", [128, 512], F16, isOutput=False)
    w4_d = nc.declare_dram_parameter("w4", [128, 8], F16, isOutput=False)
    if use_bias:
        b123_d = nc.declare_dram_parameter("b123", [128, 6], F32, isOutput=False)
        b4_d = nc.declare_dram_parameter("b4r", [4, 1], F32, isOutput=False)
    out_d = nc.declare_dram_parameter("outT", [4, n_core], F32, isOutput=True)

    chunks = [(c0, min(c0 + CHUNK_TILES, tiles))
              for c0 in range(0, tiles, CHUNK_TILES)]

    with tile.TileContext(nc) as tc, ExitStack() as ctx:
        const = ctx.enter_context(tc.tile_pool(name="const", bufs=1))
        data = ctx.enter_context(tc.tile_pool(name="data", bufs=1))
        acts = ctx.enter_context(tc.tile_pool(name="acts", bufs=3))
        tmp = ctx.enter_context(tc.tile_pool(name="tmp", bufs=2))
        ps = ctx.enter_context(tc.tile_pool(name="ps", bufs=1, space="PSUM"))

        # ---- weights: DMA straight into float32r tiles (bit-identical)
        w1r = const.tile([8, 512], F32R)
        nc.gpsimd.dma_start(out=w1r, in_=w1_d[:, :])
        w2r = const.tile([128, 512], F16)
        nc.gpsimd.dma_start(out=w2r, in_=w2_d[:, :])
        w3r = const.tile([128, 512], F16)
        nc.gpsimd.dma_start(out=w3r, in_=w3_d[:, :])
        w4r = const.tile([128, 8], F16)
        nc.gpsimd.dma_start(out=w4r, in_=w4_d[:, :])
        if use_bias:
            b123 = const.tile([128, 6], F32)
            nc.gpsimd.dma_start(out=b123, in_=b123_d[:, :])
            b4r = const.tile([4, 1], F32)
            nc.gpsimd.dma_start(out=b4r, in_=b4_d[:, :])

        # ---- the f32r matmul operand block stays resident (rows 0-3 are
        # the rounded state mirror, refreshed per eval).  The exact fp32
        # state lives in small per-tile ring tiles, initialized from the
        # mirror rows (bit-exact: the DMA wrote raw fp32 bits) and DMA'd
        # straight to the output after each tile's last eval.
        xm = data.tile([8, n_core], F32R)
        for (c0, c1) in chunks:
            nc.sync.dma_start(out=xm[:, c0 * NTILE:c1 * NTILE],
                              in_=xm_d[:, c0 * NTILE:c1 * NTILE])
        xmr = xm
        xmf = xm.bitcast(F32)

        mask_slot = [0]

        def bias_add(hp, cols):
            nc.vector.tensor_scalar_add(hp[:, 0:NTILE], hp[:, 0:NTILE],
                                        b123[:, cols[0]:cols[0] + 1])
            nc.vector.tensor_scalar_add(hp[:, NTILE:], hp[:, NTILE:],
                                        b123[:, cols[1]:cols[1] + 1])

        def stage_l1(ts0, is_partial, tag):
            h1p = ps.tile([128, W2N], F32, tag=tag)
            w1off = 256 if is_partial else 0
            for mt in range(2):
                for ch in range(NTILE // MM):
                    nc.tensor.matmul(
                        h1p[:, mt * NTILE + ch * MM: mt * NTILE + (ch + 1) * MM],
                        w1r[:, w1off + mt * 128: w1off + (mt + 1) * 128],
                        xmr[0:8, ts0 + ch * MM: ts0 + (ch + 1) * MM],
                        start=True, stop=True)
            if use_bias:
                bias_add(h1p, (0, 1))
            return h1p

        def stage_mid(hs_prev, wr, bias_cols, tag):
            hp = ps.tile([128, W2N], F32, tag=tag)
            for mt in range(2):
                for ch in range(NTILE // MM):
                    sl = slice(mt * NTILE + ch * MM, mt * NTILE + (ch + 1) * MM)
                    for kt in range(2):
                        nc.tensor.matmul(
                            hp[:, sl],
                            wr[:, kt * 256 + mt * 128: kt * 256 + (mt + 1) * 128],
                            hs_prev[:, kt * NTILE + ch * MM: kt * NTILE + (ch + 1) * MM],
                            start=(kt == 0), stop=(kt == 1))
            if use_bias:
                bias_add(hp, bias_cols)
            return hp

        def stage_silu(hp, hstag):
            hs = acts.tile([128, W2N], F16, tag=hstag, bufs=2)
            if sa > 0:
                nc.scalar.activation(hs[:, 0:sa], hp[:, 0:sa], Silu)
            if sa < W2N:
                w = W2N - sa
                q4 = tmp.tile([128, w], F32, tag="q4", bufs=3)
                nc.vector._custom_dve(ops["SILU_Q4_ANT"], out=q4,
                                      in0=hp[:, sa:W2N],
                                      s0=SILU_A0, s1=SILU_A1)
                ep = tmp.tile([128, w], F32, tag="ep", bufs=3)
                nc.vector._custom_dve(ops["SILU_EP_ANT"], out=ep, in0=q4,
                                      s0=SILU_P0, s1=SILU_P1, imm2=SILU_P2)
                nc.vector._custom_dve(ops["SILU_FIN_ANT"],
                                      out=hs[:, sa:W2N],
                                      in0=ep, in1=hp[:, sa:W2N])
            return hs

        def stage_l4(hs3, tag):
            d = ps.tile([4, NTILE], F32, tag=tag)
            for ch in range(NTILE // MM):
                for kt in range(2):
                    nc.tensor.matmul(
                        d[:, ch * MM:(ch + 1) * MM],
                        w4r[:, kt * 4:(kt + 1) * 4],
                        hs3[:, kt * NTILE + ch * MM: kt * NTILE + (ch + 1) * MM],
                        start=(kt == 0), stop=(kt == 1))
            if use_bias:
                nc.vector.tensor_scalar_add(d, d, b4r[:, 0:1])
            return d

        def stage_add(t, ts, d, use_mask, is_last):
            # in-place f32r accumulation: one DVE add per eval, no mirror
            # cast on the saturated Scalar engine and one fewer hop in the
            # serial eval chain.  Costs ~tf32 rounding of the state per
            # update (~3e-3 final scale-relative error, gate is 2e-2).
            dd = d
            if use_mask:
                j = mask_slot[0]
                mask_slot[0] += 1
                mk = tmp.tile([4, NTILE], F32, tag="mk")
                nc.sync.dma_start(out=mk, in_=mk_d[:, j * NTILE:(j + 1) * NTILE])
                dm = tmp.tile([4, NTILE], F32, tag="dm")
                nc.vector.tensor_mul(dm, d, mk)
                dd = dm
            nc.vector.tensor_add(xmr[0:4, ts], xmr[0:4, ts], dd)
            if is_last:
                nc.sync.dma_start(out=out_d[:, ts], in_=xmf[0:4, ts])

        cur_hs = {}
        for t, e, (isp, use_mask), stage in _emit_slots(schedule, tiles):
            tag = f"h{t % GROUP}"
            hstag = f"hs{t % GROUP}"
            ts = slice(t * NTILE, (t + 1) * NTILE)
            if stage == 0:
                hp = stage_l1(t * NTILE, isp, tag)
                cur_hs[t] = stage_silu(hp, hstag)
            elif stage == 1:
                hp = stage_mid(cur_hs[t], w2r, (2, 3), tag)
                cur_hs[t] = stage_silu(hp, hstag)
            elif stage == 2:
                hp = stage_mid(cur_hs[t], w3r, (4, 5), tag)
                cur_hs[t] = stage_silu(hp, hstag)
            else:
                d = stage_l4(cur_hs.pop(t), tag)
                stage_add(t, ts, d, use_mask, e == len(schedule[t]) - 1)

        for t in range(tiles):
            if len(schedule[t]) == 0:
                ts = slice(t * NTILE, (t + 1) * NTILE)
                nc.sync.dma_start(out=out_d[:, ts], in_=xmf[0:4, ts])

    nc.compile()
    return nc


def kernel(state_dz, z_frac, W1, b1, W2, b2, W3, b3, W4, b4):
    global LAST_EXEC_NS
    import os

    sa = int(os.environ.get("BASSK_SA", SA_DEFAULT))

    state_dz = np.ascontiguousarray(state_dz, dtype=np.float32)
    z_frac = np.ascontiguousarray(z_frac, dtype=np.float32)
    W1 = np.asarray(W1, np.float32); W2 = np.asarray(W2, np.float32)
    W3 = np.asarray(W3, np.float32); W4 = np.asarray(W4, np.float32)
    b1 = np.asarray(b1, np.float32); b2 = np.asarray(b2, np.float32)
    b3 = np.asarray(b3, np.float32); b4 = np.asarray(b4, np.float32)

    B = state_dz.shape[0]
    assert B % (NCORES * NTILE) == 0, f"B={B} must be divisible by {NCORES * NTILE}"
    n_core = B // NCORES
    tiles = n_core // NTILE

    # ---- host-side derived quantities (bitwise-identical fp32 ops vs jax)
    dz_sub = (state_dz[:, 5] / np.float32(8.0)).astype(np.float32)
    cont = (z_frac * np.float32(NSTEPS)).astype(np.float32)
    n_full = np.floor(cont).astype(np.float32)
    frac = (cont - n_full).astype(np.float32)
    dz_part = (frac * dz_sub).astype(np.float32)
    has_part = (frac > np.float32(1e-6)).astype(np.float32)
    n_int = np.minimum(n_full, NSTEPS).astype(np.int64)

    # ---- sort desc by n_full, deal round-robin to cores
    order = np.argsort(-n_int, kind="stable")
    perms = [order[c::NCORES] for c in range(NCORES)]

    # ---- build per-core xm arrays  [8, N_CORE]
    xms = []
    for c in range(NCORES):
        p = perms[c]
        xm = np.zeros((8, n_core), np.float32)
        xm[0:4] = state_dz[p, 0:4].T
        xm[R_QOP] = state_dz[p, 4]
        xm[R_DZSUB] = dz_sub[p]
        xm[R_DZPART] = dz_part[p]
        xms.append(xm)

    # ---- union schedule across cores (SPMD: one program for all cores)
    sched = []
    for t in range(tiles):
        sl = slice(t * NTILE, (t + 1) * NTILE)
        smax, smin = 0, NSTEPS
        anyp, allp = False, True
        for c in range(NCORES):
            nf = n_int[perms[c][sl]]
            smax = max(smax, int(nf.max()))
            smin = min(smin, int(nf.min()))
            hp = has_part[perms[c][sl]]
            anyp = anyp or bool(hp.any())
            allp = allp and bool(hp.all())
        evals = []
        for s in range(min(smax, NSTEPS)):
            evals.append((False, smin <= s))
        if anyp:
            evals.append((True, not allp))
        sched.append(tuple(evals))
    sched = tuple(sched)

    # masked evals in DEVICE EMISSION order of the add stage
    masked_evals = []
    for t, e, (is_partial, use_mask), stage in _emit_slots(sched, tiles):
        if stage == 3 and use_mask:
            masked_evals.append((t, None if is_partial else e))

    # ---- packed mask rows (broadcast to 4 partitions), per core
    nm = max(1, len(masked_evals))
    mask4s = [np.zeros((4, nm * NTILE), np.float32) for _ in range(NCORES)]
    for j, (t, s) in enumerate(masked_evals):
        sl = slice(t * NTILE, (t + 1) * NTILE)
        for c in range(NCORES):
            idx = perms[c][sl]
            row = has_part[idx] if s is None else (n_full[idx] > s).astype(np.float32)
            mask4s[c][:, j * NTILE:(j + 1) * NTILE] = row[None, :]

    use_bias = bool(np.any(b1) or np.any(b2) or np.any(b3) or np.any(b4))

    key = (sched, use_bias, n_core, sa)
    if key not in _BUILD_CACHE:
        _BUILD_CACHE[key] = _build(sched, use_bias, n_core, sa)
    nc = _BUILD_CACHE[key]

    # ---- weight tensors in lhsT layouts
    w1h = np.zeros((8, 512), np.float32)
    w1h[0:6, 0:256] = W1                      # full: state,qop,dz_sub
    w1h[0:5, 256:512] = W1[0:5]               # partial: dz slot zeroed,
    w1h[6, 256:512] = W1[5]                   # dz weight reads dz_partial row
    w2h = np.concatenate([W2[0:128], W2[128:256]], axis=1).astype(np.float16)
    w3h = np.concatenate([W3[0:128], W3[128:256]], axis=1).astype(np.float16)
    w4h = np.concatenate([W4[0:128], W4[128:256]], axis=1).astype(np.float16)

    in_map = {"w1": w1h, "w2": w2h, "w3": w3h, "w4": w4h}
    if use_bias:
        b123 = np.stack([b1[0:128], b1[128:256], b2[0:128], b2[128:256],
                         b3[0:128], b3[128:256]], axis=1).astype(np.float32)
        in_map["b123"] = b123
        in_map["b4r"] = b4.reshape(4, 1).astype(np.float32)

    in_maps = [{**in_map, "xm": xms[c], "mask4": mask4s[c]}
               for c in range(NCORES)]

    trace = os.environ.get("BASSK_TRACE") == "1" and _install_ntff_hook()
    try:
        res = run_bass_kernel_spmd(nc, in_maps, list(range(NCORES)), trace=trace)
    except Exception:
        if not trace:
            raise
        res = run_bass_kernel_spmd(nc, in_maps, list(range(NCORES)), trace=False)
    LAST_EXEC_NS = res.exec_time_ns

    out = np.empty((B, 4), np.float32)
    for c in range(NCORES):
        out[perms[c], :] = res.results[c]["outT"].T
    return out
